# revision 8
# baseline (speedup 1.0000x reference)
"""Causal multi-head attention (RoPE) for Trainium2, tensor-parallel over 8 NeuronCores.

Problem: B=2, T=2048, DM=2048, H=16 heads, D=128 head dim, fp32, causal SDPA
with rotary embeddings, y = Attention(x) @ wo^T.

Sharding: 2 heads per core (wq/wk/wv column-sharded, wo row-sharded).  Each core
computes a full [B*T, DM] partial of the output projection; the host sums the 8
partials (the all-reduce of row-parallel wo).

Per-core kernel (all matmuls in float32r — fp32 storage, TF32-like PE mode,
4x faster than strict fp32):
  Phase A  qT/kT = (wq x)^T, (wk x)^T in [d, t] layout + RoPE on DVE (4 ops);
           v in [t, d] layout.  x is consumed pre-transposed (host-prepped xT).
  Phase B  per q-block of 512 (descending), per head: S^T = K Q^T on the PE
           ([k, q] layout: no on-chip transposes anywhere), exp on ACT (scale
           folded), causal mask via gpsimd affine_select, P^T V and the
           ones-vector row-sums accumulated in PSUM.  Diagonal k-tiles run at
           partial q-width.  The kt loop is software-pipelined (PV/sums trail
           scores by two steps).  Normalization runs entirely off the PE: the
           row-sums take a DRAM round trip that transposes [1,512] -> [128,4]
           so the reciprocal uses all DVE lanes, then a broadcast-DMA fans the
           result back to [128,512] and one DVE multiply writes O^T; all of it
           is emitted one head later so no engine ever blocks on the chain.
  Phase C  y = sum_h O_h^T @ woT_h, emitted one q-block behind Phase B.
"""
import sys

sys.path.insert(0, '/opt/trn_rl_repo')

import math

import numpy as np

import concourse.bass as bass  # noqa: F401  (bass must import before bacc)
import concourse.mybir as mybir
import concourse.tile as tile
from concourse import bacc, bass_utils

B, T, DM = 2, 2048, 2048
H, D = 16, 128
HALF = D // 2
NCORES = 8
HPC = H // NCORES          # heads per core
HD = HPC * D               # per-core head-dim total (256)
P = 128
KO = DM // P               # k-tiles over the model dim (16)
TB = 512                   # phase-A t-block
QB = 512                   # attention q-block
NQB = T // QB
KPB = QB // P              # k-tiles per q-block (4)
f32 = mybir.dt.float32
f32r = mybir.dt.float32r
SCALE = 1.0 / math.sqrt(D)
EXP = mybir.ActivationFunctionType.Exp

_NC_CACHE = {}


def _build_nc():
    nc = bacc.Bacc("TRN2")
    xT_d = nc.dram_tensor("xT", [DM, B * T], f32r, kind="ExternalInput")
    wqT_d = nc.dram_tensor("wqT", [DM, HD], f32r, kind="ExternalInput")
    wkT_d = nc.dram_tensor("wkT", [DM, HD], f32r, kind="ExternalInput")
    wvT_d = nc.dram_tensor("wvT", [DM, HD], f32r, kind="ExternalInput")
    woT_d = nc.dram_tensor("woT", [HD, DM], f32r, kind="ExternalInput")
    cse_d = nc.dram_tensor("cse", [P, T], f32, kind="ExternalInput")
    sse_d = nc.dram_tensor("sse", [P, T], f32, kind="ExternalInput")
    ones_d = nc.dram_tensor("ones", [P, P], f32r, kind="ExternalInput")
    y_d = nc.dram_tensor("y", [B * T, DM], f32, kind="ExternalOutput")

    with tile.TileContext(nc) as tc:
        _body(nc, tc, xT_d, wqT_d, wkT_d, wvT_d, woT_d, cse_d, sse_d, ones_d, y_d)
    nc.compile()
    return nc


def _body(nc, tc, xT_d, wqT_d, wkT_d, wvT_d, woT_d, cse_d, sse_d, ones_d, y_d):
    with (
        tc.tile_pool(name="const", bufs=1) as const,
        tc.tile_pool(name="qkv", bufs=1) as qkv,
        tc.tile_pool(name="xt", bufs=17) as xpool,
        tc.tile_pool(name="rope", bufs=1) as rope,
        tc.tile_pool(name="pt", bufs=4) as ptpool,
        tc.tile_pool(name="ot", bufs=2) as otpool,
        tc.tile_pool(name="norm", bufs=1) as norm,
        tc.tile_pool(name="y", bufs=2) as ypool,
        tc.tile_pool(name="dram", bufs=2, space="DRAM") as dram,
        tc.tile_pool(name="ps", bufs=1, space="PSUM") as ps,
    ):
        xTr = xT_d.ap().rearrange("(ko p) t -> p ko t", p=P)
        yap = y_d.ap()

        # ---- loads: wq/wk interleaved with batch-0/block-0 xt on the sync
        # rail (these feed the first matmuls), everything else behind/gpsimd.
        w_sb = {nm: const.tile([P, KO, HD], f32r, tag=nm, name=nm)
                for nm in ("wq", "wk", "wv")}
        w_r = {nm: d.ap().rearrange("(ko p) h -> p ko h", p=P)
               for nm, d in (("wq", wqT_d), ("wk", wkT_d), ("wv", wvT_d))}
        xt0 = [xpool.tile([P, TB], f32r, tag="xt", name="xt") for _ in range(KO)]
        for ko in range(KO):
            nc.sync.dma_start(w_sb["wq"][:, ko], w_r["wq"][:, ko])
            nc.gpsimd.dma_start(w_sb["wk"][:, ko], w_r["wk"][:, ko])
            nc.sync.dma_start(xt0[ko][:], xTr[:, ko, 0:TB])
        for ko in range(KO):
            nc.sync.dma_start(w_sb["wv"][:, ko], w_r["wv"][:, ko])
        cse_sb = const.tile([P, T], f32, tag="cse")
        sse_sb = const.tile([P, T], f32, tag="sse")
        nc.gpsimd.dma_start(cse_sb[:], cse_d.ap())
        nc.gpsimd.dma_start(sse_sb[:], sse_d.ap())
        ones_sb = const.tile([P, P], f32r, tag="ones")
        nc.gpsimd.dma_start(ones_sb[:], ones_d.ap())
        wo_sb = const.tile([P, HD // P, DM], f32r, tag="wo")
        wor = woT_d.ap().rearrange("(ko p) d -> p ko d", p=P)
        for ko in range(HD // P):
            nc.gpsimd.dma_start(wo_sb[:, ko], wor[:, ko])

        def pbank(i):
            return ps.tile([P, QB], f32, tag=f"p{i}", name=f"ps_p{i}")

        # deferred-emission slots
        pend = {"epi": [], "c": []}

        def flush(which):
            fns = pend[which]
            pend[which] = []
            for fn in fns:
                fn()

        def emit_c(b, qb, ot_pair):
            def fn():
                for tt in range(KPB):
                    trow = b * T + qb * QB + tt * P
                    for db in range(DM // QB):
                        y_ps = pbank(6 if db % 2 == 0 else 7)
                        for h in range(HPC):
                            nc.tensor.matmul(y_ps, ot_pair[h][:, tt * P:(tt + 1) * P],
                                             wo_sb[:, h, db * QB:(db + 1) * QB],
                                             start=(h == 0), stop=(h == HPC - 1))
                        y_sb = ypool.tile([P, QB], f32, tag="ysb")
                        nc.vector.tensor_copy(y_sb[:], y_ps)
                        nc.sync.dma_start(yap[trow:trow + P, db * QB:(db + 1) * QB],
                                          y_sb[:])
            pend["c"].append(fn)

        for b in range(B):
            # ---------------- Phase A: projections + RoPE ----------------
            qT = [qkv.tile([P, T], f32r, tag=f"qT{h}", name=f"qT{h}")
                  for h in range(HPC)]
            kT = [qkv.tile([P, T], f32r, tag=f"kT{h}", name=f"kT{h}")
                  for h in range(HPC)]
            v_sb = [qkv.tile([P, T // P, D], f32r, tag=f"v{h}", name=f"v{h}")
                    for h in range(HPC)]
            for tb in range(T // TB):
                t0 = b * T + tb * TB
                if b == 0 and tb == 0:
                    xt = xt0
                else:
                    xt = [xpool.tile([P, TB], f32r, tag="xt", name="xt")
                          for _ in range(KO)]
                    for ko in range(KO):
                        nc.sync.dma_start(xt[ko][:], xTr[:, ko, t0:t0 + TB])
                base = 0 if tb % 2 == 0 else 4     # qk bank parity
                obase = 4 - base                    # v uses the opposite banks
                pq = [pbank(base + 0), pbank(base + 1)]
                pk = [pbank(base + 2), pbank(base + 3)]
                for ko in range(KO):
                    st, sp = (ko == 0), (ko == KO - 1)
                    for h in range(HPC):
                        nc.tensor.matmul(pq[h], w_sb["wq"][:, ko, h * D:(h + 1) * D],
                                         xt[ko][:], start=st, stop=sp)
                        nc.tensor.matmul(pk[h], w_sb["wk"][:, ko, h * D:(h + 1) * D],
                                         xt[ko][:], start=st, stop=sp)
                if b == 1 and tb == 0:
                    # leftovers from batch 0 slot in behind the qk matmuls
                    flush("epi")
                    flush("c")
                # v in [t, d] layout
                for ts in range(TB // P):
                    pv = pbank(obase + ts)[:, :HD]
                    for ko in range(KO):
                        nc.tensor.matmul(pv, xt[ko][:, ts * P:(ts + 1) * P],
                                         w_sb["wv"][:, ko],
                                         start=(ko == 0), stop=(ko == KO - 1))
                    ktg = tb * (TB // P) + ts
                    for h in range(HPC):
                        nc.scalar.copy(v_sb[h][:, ktg], pv[:, h * D:(h + 1) * D])
                # RoPE into the f32r q/k stores (4 DVE ops per tensor)
                tcol = slice(tb * TB, (tb + 1) * TB)
                for h in range(HPC):
                    for psrc, store in ((pq[h], qT[h]), (pk[h], kT[h])):
                        tmp = rope.tile([P, TB], f32, tag="rt")
                        tmp2 = rope.tile([P, TB], f32, tag="rt2")
                        nc.vector.tensor_mul(tmp2[:], psrc, cse_sb[:, tcol])
                        nc.vector.tensor_mul(tmp[0:HALF], psrc[HALF:P],
                                             sse_sb[0:HALF, tcol])
                        nc.vector.tensor_mul(tmp[HALF:P], psrc[0:HALF],
                                             sse_sb[HALF:P, tcol])
                        nc.vector.tensor_add(store[:, tcol], tmp2[:], tmp[:])

            # ------------- Phase B (+ deferred C), q-blocks descending ----
            for qb in range(NQB - 1, -1, -1):
                ot_pair = []
                for h in range(HPC):
                    ot_ps = pbank(2 + h)
                    sums = pbank(4)[0:1, :]
                    nkt = KPB * qb + KPB
                    pipe = []

                    def drain_pv(h=h, ot_ps=ot_ps, sums=sums):
                        ppt, pkt, off, pst, psp = pipe.pop(0)
                        nc.tensor.matmul(ot_ps[:, off:], v_sb[h][:, pkt],
                                         ppt[:, off:], start=pst, stop=psp)
                        nc.tensor.matmul(sums[:, off:], ones_sb[:, 0:1],
                                         ppt[:, off:], start=pst, stop=psp)
                    for kt in range(nkt):
                        # diagonal k-tiles only need q >= kt*P: partial width
                        off = max(0, kt * P - qb * QB)
                        s_ps = pbank(0 if kt % 2 == 0 else 1)
                        nc.tensor.matmul(
                            s_ps[:, off:], kT[h][:, kt * P:(kt + 1) * P],
                            qT[h][:, qb * QB + off:(qb + 1) * QB],
                            start=True, stop=True)
                        pt = ptpool.tile([P, QB], f32r, tag="pt")
                        nc.scalar.activation(pt[:, off:], s_ps[:, off:], EXP,
                                             scale=SCALE)
                        if kt >= KPB * qb:
                            # keep where global_q >= global_k
                            nc.gpsimd.affine_select(
                                out=pt[:, off:], in_=pt[:, off:],
                                compare_op=mybir.AluOpType.is_ge, fill=0.0,
                                base=0, channel_multiplier=-1,
                                pattern=[[1, QB - off]])
                        if kt == 1:
                            flush("epi")
                            if h == 1:
                                flush("c")
                        pipe.append((pt, kt, off, kt == 0, kt == nkt - 1))
                        if len(pipe) > 2:
                            drain_pv()
                    while pipe:
                        drain_pv()
                    # row sums to SBUF now; everything else deferred a head
                    sums_sb = norm.tile([1, QB], f32, tag="sums_sb")
                    nc.scalar.copy(sums_sb[:], sums)
                    o = otpool.tile([P, QB], f32r, tag=f"ot{h}", name=f"ot{h}")
                    ot_pair.append(o)

                    def epi(sums_sb=sums_sb, ot_ps=ot_ps, o=o):
                        # transpose via DRAM so the reciprocal uses all lanes,
                        # then broadcast-DMA back; no PE/ACT instructions.
                        dsum = dram.tile([1, QB], f32, tag="dsum")
                        nc.sync.dma_start(dsum[:], sums_sb[:])
                        scol = norm.tile([P, QB // P], f32, tag="scol")
                        nc.sync.dma_start(
                            scol[:], dsum[:].rearrange("a (p j) -> (a p) j", p=P))
                        rcol = norm.tile([P, QB // P], f32, tag="rcol")
                        nc.vector.reciprocal(rcol[:], scol[:])
                        drec = dram.tile([1, QB], f32, tag="drec")
                        nc.sync.dma_start(
                            drec[:].rearrange("a (p j) -> (a p) j", p=P), rcol[:])
                        bc_sb = norm.tile([P, QB], f32, tag="bcsb")
                        nc.sync.dma_start(bc_sb[:], drec[:].to_broadcast([P, QB]))
                        nc.vector.tensor_mul(o[:], ot_ps, bc_sb[:])
                    pend["epi"].append(epi)
                emit_c(b, qb, ot_pair)
        flush("epi")
        flush("c")


def _prep_inputs(x, wq, wk, wv, wo, cos, sin):
    x = np.ascontiguousarray(np.asarray(x, np.float32).reshape(B * T, DM))
    xT = np.ascontiguousarray(x.T)
    cosT = np.asarray(cos, np.float32).T
    sinT = np.asarray(sin, np.float32).T
    cse = np.ascontiguousarray(np.concatenate([cosT, cosT], axis=0))
    sse = np.ascontiguousarray(np.concatenate([-sinT, sinT], axis=0))
    ones = np.ones((P, P), np.float32)
    wq = np.asarray(wq, np.float32)
    wk = np.asarray(wk, np.float32)
    wv = np.asarray(wv, np.float32)
    wo = np.asarray(wo, np.float32)
    in_maps = []
    for c in range(NCORES):
        hs = slice(c * HD, (c + 1) * HD)
        in_maps.append({
            "xT": xT,
            "wqT": np.ascontiguousarray(wq[hs].T),
            "wkT": np.ascontiguousarray(wk[hs].T),
            "wvT": np.ascontiguousarray(wv[hs].T),
            "woT": np.ascontiguousarray(wo[:, hs].T),
            "cse": cse,
            "sse": sse,
            "ones": ones,
        })
    return in_maps


def _get_nc():
    if "nc" not in _NC_CACHE:
        _NC_CACHE["nc"] = _build_nc()
    return _NC_CACHE["nc"]


def _run(in_maps, **kwargs):
    nc = _get_nc()
    return bass_utils.run_bass_kernel_spmd(nc, in_maps,
                                           core_ids=list(range(NCORES)), **kwargs)


def kernel(x, wq, wk, wv, wo, cos, sin):
    in_maps = _prep_inputs(x, wq, wk, wv, wo, cos, sin)
    res = _run(in_maps)
    y = res.results[0]["y"].copy()
    for c in range(1, NCORES):
        y += res.results[c]["y"]
    return y.reshape(B, T, DM)


# revision 9
# speedup vs baseline: 1.1171x; 1.1171x over previous
"""Causal multi-head attention (RoPE) for Trainium2, tensor-parallel over 8 NeuronCores.

Problem: B=2, T=2048, DM=2048, H=16 heads, D=128 head dim, fp32, causal SDPA
with rotary embeddings, y = Attention(x) @ wo^T.

Sharding: 2 heads per core (wq/wk/wv column-sharded, wo row-sharded).  Each core
computes a full [B*T, DM] partial of the output projection; the host sums the 8
partials (the all-reduce of row-parallel wo).

Per-core kernel (all matmuls in float32r — fp32 storage, TF32-like PE mode,
4x faster than strict fp32).  Projection blocks and attention q-blocks are
interleaved (A0 B0 A1 B1 ...) so the DVE/ACT feeder work of each stage hides
under the other stage's PE-dense stretches and the PE never idles long enough
for the HAM clock gate to re-throttle:
  A(tb)  qT/kT = (wq x)^T, (wk x)^T in [d, t] layout + RoPE on DVE (4 ops);
         v in [t, d] layout.  x is consumed pre-transposed (host-prepped xT).
  B(qb)  per head: S^T = K Q^T on the PE ([k, q] layout: no on-chip transposes
         anywhere), exp on ACT (scale folded), causal mask via gpsimd
         affine_select, P^T V and ones-vector row-sums accumulated in PSUM.
         Diagonal k-tiles run at partial q-width.  The kt loop is software-
         pipelined (PV/sums trail scores by two steps).  Normalization runs
         entirely off the PE: the row-sums take a DRAM round trip that
         transposes [1,512] -> [128,4] so the reciprocal uses all DVE lanes,
         then a broadcast-DMA fans the result back to [128,512] and one DVE
         multiply writes O^T; all of it is emitted one head later so no
         engine ever blocks on the chain.
  C(qb)  y = sum_h O_h^T @ woT_h, emitted one q-block behind B.
"""
import sys

sys.path.insert(0, '/opt/trn_rl_repo')

import math

import numpy as np

import concourse.bass as bass  # noqa: F401  (bass must import before bacc)
import concourse.mybir as mybir
import concourse.tile as tile
from concourse import bacc, bass_utils

B, T, DM = 2, 2048, 2048
H, D = 16, 128
HALF = D // 2
NCORES = 8
HPC = H // NCORES          # heads per core
HD = HPC * D               # per-core head-dim total (256)
P = 128
KO = DM // P               # k-tiles over the model dim (16)
TB = 512                   # phase-A t-block == attention q-block
QB = 512
NQB = T // QB
KPB = QB // P              # k-tiles per q-block (4)
f32 = mybir.dt.float32
f32r = mybir.dt.float32r
SCALE = 1.0 / math.sqrt(D)
EXP = mybir.ActivationFunctionType.Exp

_NC_CACHE = {}


def _build_nc():
    nc = bacc.Bacc("TRN2")
    xT_d = nc.dram_tensor("xT", [DM, B * T], f32r, kind="ExternalInput")
    wqT_d = nc.dram_tensor("wqT", [DM, HD], f32r, kind="ExternalInput")
    wkT_d = nc.dram_tensor("wkT", [DM, HD], f32r, kind="ExternalInput")
    wvT_d = nc.dram_tensor("wvT", [DM, HD], f32r, kind="ExternalInput")
    woT_d = nc.dram_tensor("woT", [HD, DM], f32r, kind="ExternalInput")
    cse_d = nc.dram_tensor("cse", [P, T], f32, kind="ExternalInput")
    sse_d = nc.dram_tensor("sse", [P, T], f32, kind="ExternalInput")
    ones_d = nc.dram_tensor("ones", [P, P], f32r, kind="ExternalInput")
    y_d = nc.dram_tensor("y", [B * T, DM], f32, kind="ExternalOutput")

    with tile.TileContext(nc) as tc:
        _body(nc, tc, xT_d, wqT_d, wkT_d, wvT_d, woT_d, cse_d, sse_d, ones_d, y_d)
    nc.compile()
    return nc


def _body(nc, tc, xT_d, wqT_d, wkT_d, wvT_d, woT_d, cse_d, sse_d, ones_d, y_d):
    with (
        tc.tile_pool(name="const", bufs=1) as const,
        tc.tile_pool(name="qkv", bufs=1) as qkv,
        tc.tile_pool(name="xt", bufs=16) as xpool,
        tc.tile_pool(name="rope", bufs=1) as rope,
        tc.tile_pool(name="pt", bufs=5) as ptpool,
        tc.tile_pool(name="ot", bufs=2) as otpool,
        tc.tile_pool(name="norm", bufs=1) as norm,
        tc.tile_pool(name="y", bufs=2) as ypool,
        tc.tile_pool(name="dram", bufs=2, space="DRAM") as dram,
        tc.tile_pool(name="ps", bufs=1, space="PSUM") as ps,
    ):
        xTr = xT_d.ap().rearrange("(ko p) t -> p ko t", p=P)
        yap = y_d.ap()

        # ---- loads: wq (sync) / wk (gpsimd) interleaved with block-0 xt on
        # the sync rail (these feed the first matmuls); bulk behind them.
        w_sb = {nm: const.tile([P, KO, HD], f32r, tag=nm, name=nm)
                for nm in ("wq", "wk", "wv")}
        w_r = {nm: d.ap().rearrange("(ko p) h -> p ko h", p=P)
               for nm, d in (("wq", wqT_d), ("wk", wkT_d), ("wv", wvT_d))}
        xt0 = [xpool.tile([P, TB], f32r, tag="xt", name="xt") for _ in range(KO)]
        for ko in range(KO):
            nc.sync.dma_start(w_sb["wq"][:, ko], w_r["wq"][:, ko])
            nc.gpsimd.dma_start(w_sb["wk"][:, ko], w_r["wk"][:, ko])
            nc.sync.dma_start(xt0[ko][:], xTr[:, ko, 0:TB])
        for ko in range(KO):
            nc.sync.dma_start(w_sb["wv"][:, ko], w_r["wv"][:, ko])
        cse_sb = const.tile([P, T], f32, tag="cse")
        sse_sb = const.tile([P, T], f32, tag="sse")
        nc.gpsimd.dma_start(cse_sb[:], cse_d.ap())
        nc.gpsimd.dma_start(sse_sb[:], sse_d.ap())
        ones_sb = const.tile([P, P], f32r, tag="ones")
        nc.gpsimd.dma_start(ones_sb[:], ones_d.ap())
        wo_sb = const.tile([P, HD // P, DM], f32r, tag="wo")
        wor = woT_d.ap().rearrange("(ko p) d -> p ko d", p=P)
        for ko in range(HD // P):
            nc.gpsimd.dma_start(wo_sb[:, ko], wor[:, ko])

        def pbank(i):
            return ps.tile([P, QB], f32, tag=f"p{i}", name=f"ps_p{i}")

        # deferred-emission slots
        pend = {"epi": [], "c": []}

        def flush(which):
            fns = pend[which]
            pend[which] = []
            for fn in fns:
                fn()

        def emit_c(b, qb, ot_pair):
            def fn():
                for tt in range(KPB):
                    trow = b * T + qb * QB + tt * P
                    for db in range(DM // QB):
                        y_ps = pbank(3 if db % 2 == 0 else 4)
                        for h in range(HPC):
                            nc.tensor.matmul(y_ps, ot_pair[h][:, tt * P:(tt + 1) * P],
                                             wo_sb[:, h, db * QB:(db + 1) * QB],
                                             start=(h == 0), stop=(h == HPC - 1))
                        y_sb = ypool.tile([P, QB], f32, tag="ysb")
                        if (tt * (DM // QB) + db) % 2 == 0:
                            nc.scalar.copy(y_sb[:], y_ps)
                        else:
                            nc.vector.tensor_copy(y_sb[:], y_ps)
                        nc.sync.dma_start(yap[trow:trow + P, db * QB:(db + 1) * QB],
                                          y_sb[:])
            pend["c"].append(fn)

        def phase_a_block(b, tb, xt, qT, kT, v_sb):
            # qk projections: banks p0-p3; v: p4/p5
            pq = [pbank(0), pbank(1)]
            pk = [pbank(2), pbank(3)]
            for ko in range(KO):
                st, sp = (ko == 0), (ko == KO - 1)
                for h in range(HPC):
                    nc.tensor.matmul(pq[h], w_sb["wq"][:, ko, h * D:(h + 1) * D],
                                     xt[ko][:], start=st, stop=sp)
                    nc.tensor.matmul(pk[h], w_sb["wk"][:, ko, h * D:(h + 1) * D],
                                     xt[ko][:], start=st, stop=sp)
            if b == 1 and tb == 0:
                # leftovers from batch 0 slot in behind the qk matmuls
                flush("epi")
                flush("c")
            # v in [t, d] layout
            for ts in range(TB // P):
                pv = pbank(4 if ts % 2 == 0 else 5)[:, :HD]
                for ko in range(KO):
                    nc.tensor.matmul(pv, xt[ko][:, ts * P:(ts + 1) * P],
                                     w_sb["wv"][:, ko],
                                     start=(ko == 0), stop=(ko == KO - 1))
                ktg = tb * (TB // P) + ts
                for h in range(HPC):
                    nc.scalar.copy(v_sb[h][:, ktg], pv[:, h * D:(h + 1) * D])
            # RoPE into the f32r q/k stores (4 DVE ops per tensor)
            tcol = slice(tb * TB, (tb + 1) * TB)
            for h in range(HPC):
                for psrc, store in ((pq[h], qT[h]), (pk[h], kT[h])):
                    tmp = rope.tile([P, TB], f32, tag="rt")
                    tmp2 = rope.tile([P, TB], f32, tag="rt2")
                    nc.vector.tensor_mul(tmp2[:], psrc, cse_sb[:, tcol])
                    nc.vector.tensor_mul(tmp[0:HALF], psrc[HALF:P],
                                         sse_sb[0:HALF, tcol])
                    nc.vector.tensor_mul(tmp[HALF:P], psrc[0:HALF],
                                         sse_sb[HALF:P, tcol])
                    nc.vector.tensor_add(store[:, tcol], tmp2[:], tmp[:])

        def phase_b_block(b, qb, qT, kT, v_sb):
            ot_pair = []
            for h in range(HPC):
                ot_ps = pbank(0 if h == 0 else 1)
                sums = pbank(2)[0:1, :]
                nkt = KPB * qb + KPB
                pipe = []

                def drain_pv(h=h, ot_ps=ot_ps, sums=sums):
                    ppt, pkt, off, pst, psp = pipe.pop(0)
                    nc.tensor.matmul(ot_ps[:, off:], v_sb[h][:, pkt],
                                     ppt[:, off:], start=pst, stop=psp)
                    nc.tensor.matmul(sums[:, off:], ones_sb[:, 0:1],
                                     ppt[:, off:], start=pst, stop=psp)
                for kt in range(nkt):
                    # diagonal k-tiles only need q >= kt*P: partial width
                    off = max(0, kt * P - qb * QB)
                    s_ps = pbank(6 if kt % 2 == 0 else 7)
                    nc.tensor.matmul(
                        s_ps[:, off:], kT[h][:, kt * P:(kt + 1) * P],
                        qT[h][:, qb * QB + off:(qb + 1) * QB],
                        start=True, stop=True)
                    pt = ptpool.tile([P, QB], f32r, tag="pt")
                    nc.scalar.activation(pt[:, off:], s_ps[:, off:], EXP,
                                         scale=SCALE)
                    if kt >= KPB * qb:
                        # keep where global_q >= global_k
                        nc.gpsimd.affine_select(
                            out=pt[:, off:], in_=pt[:, off:],
                            compare_op=mybir.AluOpType.is_ge, fill=0.0,
                            base=0, channel_multiplier=-1,
                            pattern=[[1, QB - off]])
                    if kt == 1:
                        flush("epi")
                        if h == 1:
                            flush("c")
                    pipe.append((pt, kt, off, kt == 0, kt == nkt - 1))
                    if len(pipe) > 2:
                        drain_pv()
                while pipe:
                    drain_pv()
                # row sums to SBUF now; everything else deferred a head
                sums_sb = norm.tile([1, QB], f32, tag="sums_sb")
                nc.scalar.copy(sums_sb[:], sums)
                o = otpool.tile([P, QB], f32r, tag=f"ot{h}", name=f"ot{h}")
                ot_pair.append(o)

                def epi(sums_sb=sums_sb, ot_ps=ot_ps, o=o):
                    # transpose via DRAM so the reciprocal uses all lanes,
                    # then broadcast-DMA back; no PE/ACT instructions.
                    dsum = dram.tile([1, QB], f32, tag="dsum")
                    nc.sync.dma_start(dsum[:], sums_sb[:])
                    scol = norm.tile([P, QB // P], f32, tag="scol")
                    nc.sync.dma_start(
                        scol[:], dsum[:].rearrange("a (p j) -> (a p) j", p=P))
                    rcol = norm.tile([P, QB // P], f32, tag="rcol")
                    nc.vector.reciprocal(rcol[:], scol[:])
                    drec = dram.tile([1, QB], f32, tag="drec")
                    nc.sync.dma_start(
                        drec[:].rearrange("a (p j) -> (a p) j", p=P), rcol[:])
                    bc_sb = norm.tile([P, QB], f32, tag="bcsb")
                    nc.sync.dma_start(bc_sb[:], drec[:].to_broadcast([P, QB]))
                    nc.vector.tensor_mul(o[:], ot_ps, bc_sb[:])
                pend["epi"].append(epi)
            emit_c(b, qb, ot_pair)

        for b in range(B):
            qT = [qkv.tile([P, T], f32r, tag=f"qT{h}", name=f"qT{h}")
                  for h in range(HPC)]
            kT = [qkv.tile([P, T], f32r, tag=f"kT{h}", name=f"kT{h}")
                  for h in range(HPC)]
            v_sb = [qkv.tile([P, T // P, D], f32r, tag=f"v{h}", name=f"v{h}")
                    for h in range(HPC)]
            for tb in range(T // TB):
                t0 = b * T + tb * TB
                if b == 0 and tb == 0:
                    xt = xt0
                else:
                    xt = [xpool.tile([P, TB], f32r, tag="xt", name="xt")
                          for _ in range(KO)]
                    for ko in range(KO):
                        nc.sync.dma_start(xt[ko][:], xTr[:, ko, t0:t0 + TB])
                phase_a_block(b, tb, xt, qT, kT, v_sb)
                phase_b_block(b, tb, qT, kT, v_sb)
        flush("epi")
        flush("c")


def _prep_inputs(x, wq, wk, wv, wo, cos, sin):
    x = np.ascontiguousarray(np.asarray(x, np.float32).reshape(B * T, DM))
    xT = np.ascontiguousarray(x.T)
    cosT = np.asarray(cos, np.float32).T
    sinT = np.asarray(sin, np.float32).T
    cse = np.ascontiguousarray(np.concatenate([cosT, cosT], axis=0))
    sse = np.ascontiguousarray(np.concatenate([-sinT, sinT], axis=0))
    ones = np.ones((P, P), np.float32)
    wq = np.asarray(wq, np.float32)
    wk = np.asarray(wk, np.float32)
    wv = np.asarray(wv, np.float32)
    wo = np.asarray(wo, np.float32)
    in_maps = []
    for c in range(NCORES):
        hs = slice(c * HD, (c + 1) * HD)
        in_maps.append({
            "xT": xT,
            "wqT": np.ascontiguousarray(wq[hs].T),
            "wkT": np.ascontiguousarray(wk[hs].T),
            "wvT": np.ascontiguousarray(wv[hs].T),
            "woT": np.ascontiguousarray(wo[:, hs].T),
            "cse": cse,
            "sse": sse,
            "ones": ones,
        })
    return in_maps


def _get_nc():
    if "nc" not in _NC_CACHE:
        _NC_CACHE["nc"] = _build_nc()
    return _NC_CACHE["nc"]


def _run(in_maps, **kwargs):
    nc = _get_nc()
    return bass_utils.run_bass_kernel_spmd(nc, in_maps,
                                           core_ids=list(range(NCORES)), **kwargs)


def kernel(x, wq, wk, wv, wo, cos, sin):
    in_maps = _prep_inputs(x, wq, wk, wv, wo, cos, sin)
    res = _run(in_maps)
    y = res.results[0]["y"].copy()
    for c in range(1, NCORES):
        y += res.results[c]["y"]
    return y.reshape(B, T, DM)
